# revision 22
# baseline (speedup 1.0000x reference)
"""Additive attention (Bahdanau) Trainium2 kernel, SPMD over 8 NeuronCores.

Reference computation (per batch b):
    q = queries @ W_q                    [Q, H]
    k = keys    @ W_k                    [K, H]
    scores[q,k] = sum_h w_v[h] * tanh(q[q,h] + k[k,h])
    attn = masked_softmax(scores, valid_len)
    out = attn @ values                  [Q, Dv]

Algorithm: instead of materializing tanh(q+k) over [Q,K,H] (ACT-engine
bound), use a separable expansion

    tanh(a+b) ~ g(a) + sum_{j=1..R} f_j(a) * zk(b)^j,   zk = tanh(kb*b)

fitted offline by weighted least squares (constants hardcoded below).
The purely-q-dependent g(a) term is dropped: it shifts all scores of a
query equally and cancels in softmax.  Then

    scores[k,q] = sum_h [w_v[h] f_j(qp)] * zk(kp)^j

is R PSUM-accumulated matmuls per 128-key chunk (contraction over h),
with the k-side power ladder built in bf16 (ACT squares + DVE mults)
and the q-side f_j evaluated once per core on [128, B*32] tiles in
fp32 via fused scalar_tensor_tensor Horner chains (w_v folded into the
final per-partition scale).  The O(Q*K*H) transcendental work of the
direct algorithm disappears entirely.

Distribution: queries sharded across cores (32 per batch per core);
keys/values/weights replicated; keys truncated per batch to the valid
length (rounded up to even).  Downstream (masked exp via bias on the
boundary chunk, attn @ [V|1], divide by the ones-column row-sum) is
unchanged from the direct kernel.
"""

import numpy as np
import ml_dtypes

import concourse.bass as bass
import concourse.tile as tile
import concourse.bacc as bacc
from concourse import mybir
from concourse.bass_utils import run_bass_kernel_spmd

BF16 = mybir.dt.bfloat16
F32 = mybir.dt.float32
TANH = mybir.ActivationFunctionType.Tanh
EXP = mybir.ActivationFunctionType.Exp
SQUARE = mybir.ActivationFunctionType.Square
MULT = mybir.AluOpType.mult
ADD = mybir.AluOpType.add

B, Q, K, D, H, DV = 8, 256, 1024, 256, 128, 128
NCORES = 8
QSH = Q // NCORES
NEG_BIAS = -30000.0

# ---- separable-fit constants (R=10 powers of tanh(KB*b); f_j = deg-8 poly
# in tanh(KA*a), joint weighted LS fit with bf16-noise regularization) ----
KA = 0.6
KB = 0.6
RNK = 8
COEF = np.array([
  [1.65823617e+00, 0, -9.28701661e-01, 0, 3.80588701e-01, 0, -1.16799738e-01, 0],
  [0, -4.47007970e+00, 0, 4.25966400e+00, 0, -8.93315281e-01, 0, -7.21601922e-01],
  [-4.49460343e+00, 0, 1.42586775e+01, 0, -1.80341487e+01, 0, 8.81543899e+00, 0],
  [0, 1.42535615e+01, 0, -3.72843575e+01, 0, 2.85120872e+01, 0, -2.54281364e+00],
  [4.54267807e+00, 0, -3.89266034e+01, 0, 8.95275523e+01, 0, -5.97521258e+01, 0],
  [0, -1.76476939e+01, 0, 8.35299519e+01, 0, -1.03604574e+02, 0, 3.16361943e+01],
  [-1.55744284e+00, 0, 3.33493690e+01, 0, -1.18982937e+02, 0, 9.84135352e+01, 0],
  [0, 8.33919015e+00, 0, -5.46089045e+01, 0, 8.61565963e+01, 0, -3.54871786e+01],
  [-2.89473451e-01, 0, -6.00054703e+00, 0, 4.20587162e+01, 0, -4.35283774e+01, 0],
])

# per-j monic-chain decomposition: for odd j, f_j = alpha*M(w) + beta with
# w = za^2, M monic (no constant); for even j, f_j = za*(alpha*M(w) + beta).
_MONIC = []
for _j in range(1, RNK + 1):
    _col = COEF[:, _j - 1]
    _p = _col[0::2] if _j % 2 == 1 else _col[1::2]   # coeffs in w, ascending
    _alpha = float(_p[-1])
    _ms = [float(x) / _alpha for x in _p[1:-1]]      # ascending w^1..; apply reversed
    _MONIC.append((_alpha, float(_p[0]), _ms[::-1]))

_graph_cache: dict = {}


def _npad(vl):
    return int(min(max(2 * ((vl + 1) // 2), 2), K))


def _order(nps):
    kcs = [(n + 127) // 128 for n in nps]
    asc = sorted(range(B), key=lambda b: kcs[b])
    return asc[0:2] + sorted(asc[2:], key=lambda b: -kcs[b])


def _build(nps):
    nc = bacc.Bacc("TRN2", target_bir_lowering=False, debug=False,
                   num_devices=NCORES)
    KT = sum(nps)
    kcs = [(n + 127) // 128 for n in nps]
    order = _order(nps)
    pos = {b: i for i, b in enumerate(order)}   # F column block of batch b

    kT_d = nc.dram_tensor("kT", (D, KT), BF16, kind="ExternalInput").ap()
    v_d = nc.dram_tensor("vals", (KT, DV + 2), BF16, kind="ExternalInput").ap()
    # packed [W_q | qT | W_k] along columns: one fast DMA pair at startup
    wqk_d = nc.dram_tensor("wqk", (D, 2 * H + B * QSH), BF16,
                           kind="ExternalInput").ap()
    awbw_d = nc.dram_tensor("awbw", (H, 2 * RNK), F32, kind="ExternalInput").ap()
    bias_d = nc.dram_tensor("biasT", (128, B), F32, kind="ExternalInput").ap()
    out_d = nc.dram_tensor("out", (B, QSH, DV), F32, kind="ExternalOutput").ap()

    offs = np.concatenate([[0], np.cumsum(nps)]).astype(int)
    NQ = B * QSH   # 256 q columns per core

    with tile.TileContext(nc) as tc:
        with (
            tc.tile_pool(name="const", bufs=1) as const,
            tc.tile_pool(name="kt", bufs=8) as kt_pool,
            tc.tile_pool(name="zk", bufs=8) as zk_pool,
            tc.tile_pool(name="qside", bufs=1) as qside,
            tc.tile_pool(name="vt", bufs=sum(kcs)) as vpool,
            tc.tile_pool(name="pT", bufs=2) as ppool,
            tc.tile_pool(name="osb", bufs=3) as osb_pool,
            tc.tile_pool(name="proj_ps", bufs=2, space="PSUM") as proj_ps,
            tc.tile_pool(name="qp_ps", bufs=1, space="PSUM") as qp_ps_pool,
            tc.tile_pool(name="sc_ps", bufs=2, space="PSUM") as sc_ps_pool,
            tc.tile_pool(name="out_ps", bufs=3, space="PSUM") as out_ps_pool,
        ):
            # ---- packed weights+queries DMA first on the gpsimd queue
            # (the F build is on the critical path); kT on the sync queue.
            PW = 2 * H + NQ
            wqk_sb = const.tile([128, 2, PW], BF16, tag="wqk")
            nc.gpsimd.dma_start(wqk_sb[:, 0, :], wqk_d[0:128, :])
            nc.gpsimd.dma_start(wqk_sb[:, 1, :], wqk_d[128:256, :])
            awbw_sb = const.tile([H, 2 * RNK], F32, tag="awbw")
            nc.gpsimd.dma_start(awbw_sb[:], awbw_d[:, :])
            bias_sb = const.tile([128, B], F32, tag="bias")
            nc.gpsimd.dma_start(bias_sb[:], bias_d[:, :])

            dmad = {}

            def dma_a(b, first=False):
                n, off = nps[b], offs[b]
                eng = nc.sync
                kT0 = kt_pool.tile([128, n], BF16, tag="kT0")
                kT1 = kt_pool.tile([128, n], BF16, tag="kT1")
                eng.dma_start(kT0[:], kT_d[0:128, off:off + n])
                eng.dma_start(kT1[:], kT_d[128:256, off:off + n])
                vt_b = []
                for c in range((n + 127) // 128):
                    m = min(128, n - c * 128)
                    vt = vpool.tile([128, DV + 2], BF16, tag="vt")
                    r0 = off + c * 128
                    nc.gpsimd.dma_start(vt[:m, :], v_d[r0:r0 + m, :])
                    vt_b.append((vt, m))
                dmad[b] = (kT0, kT1, vt_b)

            for bb in order:
                dma_a(bb)

            # ---- q-side: projection, za, w, Horner chains -> F tiles ----
            qp_ps = qp_ps_pool.tile([128, NQ], F32, tag="qps")
            nc.tensor.matmul(qp_ps[:], wqk_sb[:, 0, 0:H], wqk_sb[:, 0, H:H + NQ],
                             start=True, stop=False)
            nc.tensor.matmul(qp_ps[:], wqk_sb[:, 1, 0:H], wqk_sb[:, 1, H:H + NQ],
                             start=False, stop=True)
            za = qside.tile([128, NQ], F32, tag="za")
            nc.scalar.activation(za[:], qp_ps[:], TANH, scale=KA)
            wsq = qside.tile([128, NQ], F32, tag="w")
            nc.scalar.activation(wsq[:], za[:], SQUARE)
            F_sb = qside.tile([128, RNK, NQ], BF16, tag="F")
            acc_a = qside.tile([128, NQ], F32, tag="acca")
            acc_b = qside.tile([128, NQ], F32, tag="accb")

            def emit_chain(j, c0, c1):
                alpha, beta, ms = _MONIC[j - 1]
                w_s, za_s = wsq[:, c0:c1], za[:, c0:c1]
                cur, nxt = acc_a[:, c0:c1], acc_b[:, c0:c1]
                nc.vector.scalar_tensor_tensor(
                    cur, w_s, ms[0], w_s, op0=ADD, op1=MULT)
                for mcoef in ms[1:]:
                    nc.vector.scalar_tensor_tensor(
                        nxt, cur, mcoef, w_s, op0=ADD, op1=MULT)
                    cur, nxt = nxt, cur
                aw = awbw_sb[:, 2 * (j - 1):2 * (j - 1) + 1]
                bw = awbw_sb[:, 2 * (j - 1) + 1:2 * (j - 1) + 2]
                if j % 2 == 1:
                    nc.vector.tensor_scalar(F_sb[:, j - 1, c0:c1], cur, aw, bw,
                                            op0=MULT, op1=ADD)
                else:
                    nc.vector.tensor_scalar(nxt, cur, aw, bw,
                                            op0=MULT, op1=ADD)
                    nc.vector.tensor_tensor(F_sb[:, j - 1, c0:c1], nxt, za_s,
                                            op=MULT)

            # ---- k-side: PE projections + ACT-only ladder passes for ALL
            # batches hoisted to phase 1 (fills PE/ACT while the DVE builds
            # the F chains); the DVE mults trail per batch in the main loop.
            zkd = {}

            def ladder_proj(b):
                n = nps[b]
                kT0, kT1, vt_b = dmad.pop(b)
                Z = zk_pool.tile([128, RNK, n], BF16, tag="Z")
                for j0 in range(0, n, 512):
                    w = min(512, n - j0)
                    ps = proj_ps.tile([128, w], F32, tag="kps")
                    nc.tensor.matmul(ps[:], wqk_sb[:, 0, H + NQ:PW], kT0[:, j0:j0 + w],
                                     start=True, stop=False)
                    nc.tensor.matmul(ps[:], wqk_sb[:, 1, H + NQ:PW], kT1[:, j0:j0 + w],
                                     start=False, stop=True)
                    nc.scalar.activation(Z[:, 0, j0:j0 + w], ps[:], TANH,
                                         scale=KB)
                z = lambda j: Z[:, j - 1, :]
                nc.scalar.activation(z(2), z(1), SQUARE)
                nc.scalar.activation(z(4), z(2), SQUARE)
                zkd[b] = (Z, vt_b)

            def ladder_muls(b):
                Z, _ = zkd[b]
                z = lambda j: Z[:, j - 1, :]
                nc.vector.tensor_tensor(z(3), z(1), z(2), op=MULT)
                nc.vector.tensor_tensor(z(5), z(1), z(4), op=MULT)
                # Z6, Z8 = Square([Z3, Z4]) in one strided ACT call
                nc.scalar.activation(Z[:, 5:8:2, :], Z[:, 2:4, :], SQUARE)
                nc.vector.tensor_tensor(z(7), z(3), z(4), op=MULT)

            for bb in order:
                ladder_proj(bb)
            # chains for the first four batches' F columns complete first
            HQ = NQ // 2
            for j in range(1, RNK + 1):
                emit_chain(j, 0, HQ)
            ladder_muls(order[0])
            ladder_muls(order[1])
            for j in range(1, RNK + 1):
                emit_chain(j, HQ, NQ)

            # ---- per-batch main loop ----
            pend_exp = None
            div_q = []
            for bi, b in enumerate(order):
                n = nps[b]
                kcb = kcs[b]
                m_last = n - (kcb - 1) * 128
                Z, vt_b = zkd.pop(b)
                sc = sc_ps_pool.tile([128, kcb * QSH], F32, tag="sc")
                if m_last < 128:
                    m0 = (m_last // 32) * 32
                    for p0 in range(m0, 128, 32):
                        nc.vector.memset(sc[p0:p0 + 32, (kcb - 1) * QSH:], 0.0)
                for c in range(kcb):
                    m = min(128, n - c * 128)
                    for j in range(1, RNK + 1):
                        nc.tensor.matmul(
                            sc[:m, c * QSH:(c + 1) * QSH],
                            Z[:, j - 1, c * 128:c * 128 + m],
                            F_sb[:, j - 1, pos[b] * QSH:(pos[b] + 1) * QSH],
                            start=(j == 1), stop=(j == RNK))
                    if c == 0:
                        if pend_exp is not None:
                            pend_exp()
                            pend_exp = None
                        if div_q and len(div_q) >= 2:
                            div_q.pop(0)()
                        if bi + 2 < B:
                            ladder_muls(order[bi + 2])
                state = {}

                def make_exp_final(b=b, kcb=kcb, sc=sc, vt_b=vt_b,
                                   state=state):
                    def exp_final():
                        pT = ppool.tile([128, kcb * QSH], BF16, tag="pT")
                        last0 = (kcb - 1) * QSH
                        if kcb > 1:
                            nc.scalar.activation(pT[:, 0:last0],
                                                 sc[:, 0:last0], EXP)
                        nc.scalar.activation(pT[:, last0:], sc[:, last0:],
                                             EXP, bias=bias_sb[:, b:b + 1])
                        ops = out_ps_pool.tile([QSH, DV + 1], F32, tag="ops")
                        for c in range(kcb):
                            vt, m = vt_b[c]
                            nc.tensor.matmul(ops[:],
                                             pT[:m, c * QSH:(c + 1) * QSH],
                                             vt[:m, 0:DV + 1],
                                             start=(c == 0),
                                             stop=(c == kcb - 1))
                        state["ops"] = ops
                    return exp_final

                def make_div(b=b, state=state):
                    def div():
                        ops = state["ops"]
                        r = osb_pool.tile([QSH, 1], F32, tag="r")
                        nc.vector.reciprocal(r[:], ops[:, DV:DV + 1])
                        osb = osb_pool.tile([QSH, DV], F32, tag="osb")
                        nc.vector.tensor_scalar_mul(osb[:], ops[:, 0:DV], r[:])
                        nc.sync.dma_start(out_d[b, :, :], osb[:])
                    return div

                pend_exp = make_exp_final()
                div_q.append(make_div())
            pend_exp()
            for dv in div_q:
                dv()
    nc.compile()
    return nc


def _prep(queries, keys, values, valid_lens):
    vl = np.asarray(valid_lens).astype(np.int64)
    nps = tuple(_npad(int(l)) for l in vl)
    KT = sum(nps)

    kT = np.empty((D, KT), ml_dtypes.bfloat16)
    vals = np.zeros((KT, DV + 2), ml_dtypes.bfloat16)
    biasT = np.zeros((128, B), np.float32)
    off = 0
    for b in range(B):
        n = nps[b]
        kT[:, off:off + n] = keys[b, :n, :].T.astype(ml_dtypes.bfloat16)
        vals[off:off + n, 0:DV] = values[b, :n, :].astype(ml_dtypes.bfloat16)
        vals[off:off + n, DV] = ml_dtypes.bfloat16(1.0)
        kcb = (n + 127) // 128
        j = np.arange(128)
        valid = (kcb - 1) * 128 + j < vl[b]
        biasT[:, b] = np.where(valid, 0.0, NEG_BIAS).astype(np.float32)
        off += n

    order = _order(nps)
    qT_shards = []
    for i in range(NCORES):
        qt = np.empty((D, B * QSH), ml_dtypes.bfloat16)
        for p, b in enumerate(order):
            qt[:, p * QSH:(p + 1) * QSH] = \
                queries[b, i * QSH:(i + 1) * QSH, :].T.astype(ml_dtypes.bfloat16)
        qT_shards.append(qt)
    return nps, kT, vals, biasT, qT_shards


def run(queries, keys, values, valid_lens, W_q, W_k, w_v, **run_kwargs):
    queries = np.asarray(queries, np.float32)
    keys = np.asarray(keys, np.float32)
    values = np.asarray(values, np.float32)
    W_q = np.asarray(W_q, np.float32)
    W_k = np.asarray(W_k, np.float32)
    w_v = np.asarray(w_v, np.float32)

    nps, kT, vals, biasT, qT_shards = _prep(queries, keys, values, valid_lens)
    awbw = np.empty((H, 2 * RNK), np.float32)
    for j in range(1, RNK + 1):
        alpha, beta, _ = _MONIC[j - 1]
        awbw[:, 2 * (j - 1)] = alpha * w_v
        awbw[:, 2 * (j - 1) + 1] = beta * w_v
    common = {
        "kT": np.ascontiguousarray(kT),
        "vals": np.ascontiguousarray(vals),
        "awbw": np.ascontiguousarray(awbw),
        "biasT": np.ascontiguousarray(biasT),
    }
    wq_bf = W_q.astype(ml_dtypes.bfloat16)
    wk_bf = W_k.astype(ml_dtypes.bfloat16)
    in_maps = [
        dict(common, wqk=np.ascontiguousarray(
            np.concatenate([wq_bf, q, wk_bf], axis=1)))
        for q in qT_shards
    ]

    nc = _graph_cache.get(nps)
    if nc is None:
        nc = _build(nps)
        _graph_cache[nps] = nc
    res = run_bass_kernel_spmd(nc, in_maps, core_ids=list(range(NCORES)),
                               **run_kwargs)
    out = np.empty((B, Q, DV), np.float32)
    for i in range(NCORES):
        out[:, i * QSH:(i + 1) * QSH, :] = res.results[i]["out"]
    return out, res


def kernel(queries, keys, values, valid_lens, W_q, W_k, w_v):
    out, _ = run(queries, keys, values, valid_lens, W_q, W_k, w_v)
    return out


# revision 23
# speedup vs baseline: 1.0617x; 1.0617x over previous
"""Additive attention (Bahdanau) Trainium2 kernel, SPMD over 8 NeuronCores.

Reference computation (per batch b):
    q = queries @ W_q                    [Q, H]
    k = keys    @ W_k                    [K, H]
    scores[q,k] = sum_h w_v[h] * tanh(q[q,h] + k[k,h])
    attn = masked_softmax(scores, valid_len)
    out = attn @ values                  [Q, Dv]

Algorithm: instead of materializing tanh(q+k) over [Q,K,H] (ACT-engine
bound), use a separable expansion

    tanh(a+b) ~ g(a) + sum_{j=1..R} f_j(a) * zk(b)^j,   zk = tanh(kb*b)

fitted offline by weighted least squares (constants hardcoded below).
The purely-q-dependent g(a) term is dropped: it shifts all scores of a
query equally and cancels in softmax.  Then

    scores[k,q] = sum_h [w_v[h] f_j(qp)] * zk(kp)^j

is R PSUM-accumulated matmuls per 128-key chunk (contraction over h),
with the k-side power ladder built in bf16 (ACT squares + DVE mults)
and the q-side f_j evaluated once per core on [128, B*32] tiles in
fp32 via fused scalar_tensor_tensor Horner chains (w_v folded into the
final per-partition scale).  The O(Q*K*H) transcendental work of the
direct algorithm disappears entirely.

Distribution: queries sharded across cores (32 per batch per core);
keys/values/weights replicated; keys truncated per batch to the valid
length (rounded up to even).  Downstream (masked exp via bias on the
boundary chunk, attn @ [V|1], divide by the ones-column row-sum) is
unchanged from the direct kernel.
"""

import numpy as np
import ml_dtypes

import concourse.bass as bass
import concourse.tile as tile
import concourse.bacc as bacc
from concourse import mybir
from concourse.bass_utils import run_bass_kernel_spmd

BF16 = mybir.dt.bfloat16
F32 = mybir.dt.float32
TANH = mybir.ActivationFunctionType.Tanh
EXP = mybir.ActivationFunctionType.Exp
SQUARE = mybir.ActivationFunctionType.Square
MULT = mybir.AluOpType.mult
ADD = mybir.AluOpType.add

B, Q, K, D, H, DV = 8, 256, 1024, 256, 128, 128
NCORES = 8
QSH = Q // NCORES
NEG_BIAS = -30000.0

# ---- separable-fit constants (R=10 powers of tanh(KB*b); f_j = deg-8 poly
# in tanh(KA*a), joint weighted LS fit with bf16-noise regularization) ----
KA = 0.6
KB = 0.6
RNK = 8
COEF = np.array([
  [1.65823617e+00, 0, -9.28701661e-01, 0, 3.80588701e-01, 0, -1.16799738e-01, 0],
  [0, -4.47007970e+00, 0, 4.25966400e+00, 0, -8.93315281e-01, 0, -7.21601922e-01],
  [-4.49460343e+00, 0, 1.42586775e+01, 0, -1.80341487e+01, 0, 8.81543899e+00, 0],
  [0, 1.42535615e+01, 0, -3.72843575e+01, 0, 2.85120872e+01, 0, -2.54281364e+00],
  [4.54267807e+00, 0, -3.89266034e+01, 0, 8.95275523e+01, 0, -5.97521258e+01, 0],
  [0, -1.76476939e+01, 0, 8.35299519e+01, 0, -1.03604574e+02, 0, 3.16361943e+01],
  [-1.55744284e+00, 0, 3.33493690e+01, 0, -1.18982937e+02, 0, 9.84135352e+01, 0],
  [0, 8.33919015e+00, 0, -5.46089045e+01, 0, 8.61565963e+01, 0, -3.54871786e+01],
  [-2.89473451e-01, 0, -6.00054703e+00, 0, 4.20587162e+01, 0, -4.35283774e+01, 0],
])

# per-j monic-chain decomposition: for odd j, f_j = alpha*M(w) + beta with
# w = za^2, M monic (no constant); for even j, f_j = za*(alpha*M(w) + beta).
_MONIC = []
for _j in range(1, RNK + 1):
    _col = COEF[:, _j - 1]
    _p = _col[0::2] if _j % 2 == 1 else _col[1::2]   # coeffs in w, ascending
    _alpha = float(_p[-1])
    _ms = [float(x) / _alpha for x in _p[1:-1]]      # ascending w^1..; apply reversed
    _MONIC.append((_alpha, float(_p[0]), _ms[::-1]))

_graph_cache: dict = {}


def _npad(vl):
    return int(min(max(2 * ((vl + 1) // 2), 2), K))


def _order(nps):
    kcs = [(n + 127) // 128 for n in nps]
    asc = sorted(range(B), key=lambda b: kcs[b])
    return asc[0:2] + sorted(asc[2:], key=lambda b: -kcs[b])


def _build(nps):
    nc = bacc.Bacc("TRN2", target_bir_lowering=False, debug=False,
                   num_devices=NCORES)
    KT = sum(nps)
    kcs = [(n + 127) // 128 for n in nps]
    order = _order(nps)
    pos = {b: i for i, b in enumerate(order)}   # F column block of batch b

    kT_d = nc.dram_tensor("kT", (D, KT), BF16, kind="ExternalInput").ap()
    v_d = nc.dram_tensor("vals", (KT, DV + 2), BF16, kind="ExternalInput").ap()
    # packed [W_q | qT | W_k] along columns: one fast DMA pair at startup
    wqk_d = nc.dram_tensor("wqk", (D, 2 * H + B * QSH), BF16,
                           kind="ExternalInput").ap()
    awbw_d = nc.dram_tensor("awbw", (H, 2 * RNK), F32, kind="ExternalInput").ap()
    bias_d = nc.dram_tensor("biasT", (128, B), F32, kind="ExternalInput").ap()
    out_d = nc.dram_tensor("out", (B, QSH, DV), F32, kind="ExternalOutput").ap()

    offs = np.concatenate([[0], np.cumsum(nps)]).astype(int)
    NQ = B * QSH   # 256 q columns per core

    with tile.TileContext(nc) as tc:
        with (
            tc.tile_pool(name="const", bufs=1) as const,
            tc.tile_pool(name="kt", bufs=8) as kt_pool,
            tc.tile_pool(name="zk", bufs=8) as zk_pool,
            tc.tile_pool(name="qside", bufs=1) as qside,
            tc.tile_pool(name="vt", bufs=sum(kcs)) as vpool,
            tc.tile_pool(name="pT", bufs=2) as ppool,
            tc.tile_pool(name="osb", bufs=3) as osb_pool,
            tc.tile_pool(name="proj_ps", bufs=2, space="PSUM") as proj_ps,
            tc.tile_pool(name="qp_ps", bufs=1, space="PSUM") as qp_ps_pool,
            tc.tile_pool(name="sc_ps", bufs=2, space="PSUM") as sc_ps_pool,
            tc.tile_pool(name="out_ps", bufs=3, space="PSUM") as out_ps_pool,
        ):
            # ---- packed weights+queries DMA first on the gpsimd queue
            # (the F build is on the critical path); kT on the sync queue.
            PW = 2 * H + NQ
            wqk_sb = const.tile([128, 2, PW], BF16, tag="wqk")
            nc.gpsimd.dma_start(wqk_sb[:, 0, :], wqk_d[0:128, :])
            nc.gpsimd.dma_start(wqk_sb[:, 1, :], wqk_d[128:256, :])
            awbw_sb = const.tile([H, 2 * RNK], F32, tag="awbw")
            nc.gpsimd.dma_start(awbw_sb[:], awbw_d[:, :])
            bias_sb = const.tile([128, B], F32, tag="bias")
            nc.gpsimd.dma_start(bias_sb[:], bias_d[:, :])

            dmad = {}

            def dma_a(b, first=False):
                n, off = nps[b], offs[b]
                eng = nc.sync
                kT0 = kt_pool.tile([128, n], BF16, tag="kT0")
                kT1 = kt_pool.tile([128, n], BF16, tag="kT1")
                eng.dma_start(kT0[:], kT_d[0:128, off:off + n])
                eng.dma_start(kT1[:], kT_d[128:256, off:off + n])
                vt_b = []
                for c in range((n + 127) // 128):
                    m = min(128, n - c * 128)
                    vt = vpool.tile([128, DV + 2], BF16, tag="vt")
                    r0 = off + c * 128
                    nc.gpsimd.dma_start(vt[:m, :], v_d[r0:r0 + m, :])
                    vt_b.append((vt, m))
                dmad[b] = (kT0, kT1, vt_b)

            for bb in order:
                dma_a(bb)

            # ---- q-side: projection, za, w, Horner chains -> F tiles ----
            qp_ps = qp_ps_pool.tile([128, NQ], F32, tag="qps")
            nc.tensor.matmul(qp_ps[:], wqk_sb[:, 0, 0:H], wqk_sb[:, 0, H:H + NQ],
                             start=True, stop=False)
            nc.tensor.matmul(qp_ps[:], wqk_sb[:, 1, 0:H], wqk_sb[:, 1, H:H + NQ],
                             start=False, stop=True)
            za = qside.tile([128, NQ], F32, tag="za")
            nc.scalar.activation(za[:], qp_ps[:], TANH, scale=KA)
            wsq = qside.tile([128, NQ], F32, tag="w")
            nc.scalar.activation(wsq[:], za[:], SQUARE)
            F_sb = qside.tile([128, RNK, NQ], BF16, tag="F")
            acc_a = qside.tile([128, NQ], F32, tag="acca")
            acc_b = qside.tile([128, NQ], F32, tag="accb")

            def emit_chain(j):
                alpha, beta, ms = _MONIC[j - 1]
                cur, nxt = acc_a, acc_b
                nc.vector.scalar_tensor_tensor(
                    cur[:], wsq[:], ms[0], wsq[:], op0=ADD, op1=MULT)
                for mcoef in ms[1:]:
                    nc.vector.scalar_tensor_tensor(
                        nxt[:], cur[:], mcoef, wsq[:], op0=ADD, op1=MULT)
                    cur, nxt = nxt, cur
                aw = awbw_sb[:, 2 * (j - 1):2 * (j - 1) + 1]
                bw = awbw_sb[:, 2 * (j - 1) + 1:2 * (j - 1) + 2]
                if j % 2 == 1:
                    nc.vector.tensor_scalar(F_sb[:, j - 1, :], cur[:], aw, bw,
                                            op0=MULT, op1=ADD)
                else:
                    nc.vector.tensor_scalar(nxt[:], cur[:], aw, bw,
                                            op0=MULT, op1=ADD)
                    nc.vector.tensor_tensor(F_sb[:, j - 1, :], nxt[:], za[:],
                                            op=MULT)

            # ---- k-side: PE projections + ACT-only ladder passes for ALL
            # batches hoisted to phase 1 (fills PE/ACT while the DVE builds
            # the F chains); the DVE mults trail per batch in the main loop.
            zkd = {}

            def ladder_proj(b):
                n = nps[b]
                kT0, kT1, vt_b = dmad.pop(b)
                Z = zk_pool.tile([128, RNK, n], BF16, tag="Z")
                for j0 in range(0, n, 512):
                    w = min(512, n - j0)
                    ps = proj_ps.tile([128, w], F32, tag="kps")
                    nc.tensor.matmul(ps[:], wqk_sb[:, 0, H + NQ:PW], kT0[:, j0:j0 + w],
                                     start=True, stop=False)
                    nc.tensor.matmul(ps[:], wqk_sb[:, 1, H + NQ:PW], kT1[:, j0:j0 + w],
                                     start=False, stop=True)
                    nc.scalar.activation(Z[:, 0, j0:j0 + w], ps[:], TANH,
                                         scale=KB)
                z = lambda j: Z[:, j - 1, :]
                nc.scalar.activation(z(2), z(1), SQUARE)
                nc.scalar.activation(z(4), z(2), SQUARE)
                zkd[b] = (Z, vt_b)

            def ladder_muls(b):
                Z, _ = zkd[b]
                z = lambda j: Z[:, j - 1, :]
                nc.vector.tensor_tensor(z(3), z(1), z(2), op=MULT)
                nc.vector.tensor_tensor(z(5), z(1), z(4), op=MULT)
                # Z6, Z8 = Square([Z3, Z4]) in one strided ACT call
                nc.scalar.activation(Z[:, 5:8:2, :], Z[:, 2:4, :], SQUARE)
                nc.vector.tensor_tensor(z(7), z(3), z(4), op=MULT)

            for bb in order:
                ladder_proj(bb)
            for j in range(1, RNK + 1):
                emit_chain(j)
            ladder_muls(order[0])
            ladder_muls(order[1])

            # ---- per-batch main loop ----
            pend_exp = None
            div_q = []
            for bi, b in enumerate(order):
                n = nps[b]
                kcb = kcs[b]
                m_last = n - (kcb - 1) * 128
                Z, vt_b = zkd.pop(b)
                sc = sc_ps_pool.tile([128, kcb * QSH], F32, tag="sc")
                if m_last < 128:
                    m0 = (m_last // 32) * 32
                    for p0 in range(m0, 128, 32):
                        nc.vector.memset(sc[p0:p0 + 32, (kcb - 1) * QSH:], 0.0)
                for c in range(kcb):
                    m = min(128, n - c * 128)
                    for j in range(1, RNK + 1):
                        nc.tensor.matmul(
                            sc[:m, c * QSH:(c + 1) * QSH],
                            Z[:, j - 1, c * 128:c * 128 + m],
                            F_sb[:, j - 1, b * QSH:(b + 1) * QSH],
                            start=(j == 1), stop=(j == RNK))
                    if c == 0:
                        if pend_exp is not None:
                            pend_exp()
                            pend_exp = None
                        if div_q and len(div_q) >= 2:
                            div_q.pop(0)()
                        if bi + 2 < B:
                            ladder_muls(order[bi + 2])
                state = {}

                def make_exp_final(b=b, kcb=kcb, sc=sc, vt_b=vt_b,
                                   state=state):
                    def exp_final():
                        pT = ppool.tile([128, kcb * QSH], BF16, tag="pT")
                        last0 = (kcb - 1) * QSH
                        if kcb > 1:
                            nc.scalar.activation(pT[:, 0:last0],
                                                 sc[:, 0:last0], EXP)
                        nc.scalar.activation(pT[:, last0:], sc[:, last0:],
                                             EXP, bias=bias_sb[:, b:b + 1])
                        ops = out_ps_pool.tile([QSH, DV + 1], F32, tag="ops")
                        for c in range(kcb):
                            vt, m = vt_b[c]
                            nc.tensor.matmul(ops[:],
                                             pT[:m, c * QSH:(c + 1) * QSH],
                                             vt[:m, 0:DV + 1],
                                             start=(c == 0),
                                             stop=(c == kcb - 1))
                        state["ops"] = ops
                    return exp_final

                def make_div(b=b, state=state):
                    def div():
                        ops = state["ops"]
                        r = osb_pool.tile([QSH, 1], F32, tag="r")
                        nc.vector.reciprocal(r[:], ops[:, DV:DV + 1])
                        osb = osb_pool.tile([QSH, DV], F32, tag="osb")
                        nc.vector.tensor_scalar_mul(osb[:], ops[:, 0:DV], r[:])
                        nc.sync.dma_start(out_d[b, :, :], osb[:])
                    return div

                pend_exp = make_exp_final()
                div_q.append(make_div())
            pend_exp()
            for dv in div_q:
                dv()
    nc.compile()
    return nc


def _prep(queries, keys, values, valid_lens):
    vl = np.asarray(valid_lens).astype(np.int64)
    nps = tuple(_npad(int(l)) for l in vl)
    KT = sum(nps)

    kT = np.empty((D, KT), ml_dtypes.bfloat16)
    vals = np.zeros((KT, DV + 2), ml_dtypes.bfloat16)
    biasT = np.zeros((128, B), np.float32)
    off = 0
    for b in range(B):
        n = nps[b]
        kT[:, off:off + n] = keys[b, :n, :].T.astype(ml_dtypes.bfloat16)
        vals[off:off + n, 0:DV] = values[b, :n, :].astype(ml_dtypes.bfloat16)
        vals[off:off + n, DV] = ml_dtypes.bfloat16(1.0)
        kcb = (n + 127) // 128
        j = np.arange(128)
        valid = (kcb - 1) * 128 + j < vl[b]
        biasT[:, b] = np.where(valid, 0.0, NEG_BIAS).astype(np.float32)
        off += n

    qT_shards = []
    for i in range(NCORES):
        qt = np.empty((D, B * QSH), ml_dtypes.bfloat16)
        for b in range(B):
            qt[:, b * QSH:(b + 1) * QSH] = \
                queries[b, i * QSH:(i + 1) * QSH, :].T.astype(ml_dtypes.bfloat16)
        qT_shards.append(qt)
    return nps, kT, vals, biasT, qT_shards


def run(queries, keys, values, valid_lens, W_q, W_k, w_v, **run_kwargs):
    queries = np.asarray(queries, np.float32)
    keys = np.asarray(keys, np.float32)
    values = np.asarray(values, np.float32)
    W_q = np.asarray(W_q, np.float32)
    W_k = np.asarray(W_k, np.float32)
    w_v = np.asarray(w_v, np.float32)

    nps, kT, vals, biasT, qT_shards = _prep(queries, keys, values, valid_lens)
    awbw = np.empty((H, 2 * RNK), np.float32)
    for j in range(1, RNK + 1):
        alpha, beta, _ = _MONIC[j - 1]
        awbw[:, 2 * (j - 1)] = alpha * w_v
        awbw[:, 2 * (j - 1) + 1] = beta * w_v
    common = {
        "kT": np.ascontiguousarray(kT),
        "vals": np.ascontiguousarray(vals),
        "awbw": np.ascontiguousarray(awbw),
        "biasT": np.ascontiguousarray(biasT),
    }
    wq_bf = W_q.astype(ml_dtypes.bfloat16)
    wk_bf = W_k.astype(ml_dtypes.bfloat16)
    in_maps = [
        dict(common, wqk=np.ascontiguousarray(
            np.concatenate([wq_bf, q, wk_bf], axis=1)))
        for q in qT_shards
    ]

    nc = _graph_cache.get(nps)
    if nc is None:
        nc = _build(nps)
        _graph_cache[nps] = nc
    res = run_bass_kernel_spmd(nc, in_maps, core_ids=list(range(NCORES)),
                               **run_kwargs)
    out = np.empty((B, Q, DV), np.float32)
    for i in range(NCORES):
        out[:, i * QSH:(i + 1) * QSH, :] = res.results[i]["out"]
    return out, res


def kernel(queries, keys, values, valid_lens, W_q, W_k, w_v):
    out, _ = run(queries, keys, values, valid_lens, W_q, W_k, w_v)
    return out
